# revision 14
# baseline (speedup 1.0000x reference)
"""Trainium2 Bass kernel for nn_EnergyToRateConverter.

Computes Eyring rates  fwd = pref*exp(-(bar - G_from)/RT),
rev = reversible ? pref*exp(-(bar - G_to)/RT) : 0  for B=1M batch rows.

Strategy (pure data parallel over 8 cores, batch split 8 ways):
  * Host transposes inputs into one feature-major fp16 tensor
    X = [state.T; (barrier - C).T] of shape (80, B).  Subtracting the
    barrier mean C (~40) first puts barriers in the same fp16 binade as
    the state energies, so a single fp16 pass already hits ~1.3e-2
    worst-case relative error (gate is 2e-2) without a second
    residual-correction matmul pass.
  * One constant matmul W.T @ X per 512-column chunk fuses the
    per-transition gather AND the barrier subtraction:
        W[from_idx[j], j] = 1 (fwd cols) / W[to_idx[j], j] = 1 (rev)
        W[32+j, j] = -1  (subtract barrier j)
    Output rows are [48 fwd | n_rev rev] with no padding; rates for
    non-reversible transitions are never computed.
  * ScalarE evaluates out = exp(psum*inv_rt + (ln(pref) - C*inv_rt))
    straight from PSUM, writing bf16 (exponent range of f32, 2^-9
    rounding) — halving output DMA bytes vs f32.
  * Input DMAs alternate the SP and Pool HWDGE queues, output DMAs ride
    the otherwise-idle DVE queue, so the ACT sequencer only runs the
    activations and no queue head-of-line-blocks another.
"""

import os

import numpy as np

B = 1048576
N_CORES = 8
BC = B // N_CORES  # 131072 batch rows per core
NS = 32
NT = 48
K = NS + NT  # 80 contraction rows: states then shifted barriers

F_SUPER = 8192  # batch columns per DMA super-tile (16KB/partition fp16)
F_PSUM = 2048  # batch columns per PSUM tile / ACT op (4 banks)
F_MM = 512  # batch columns per matmul (PE moving-dim max, one bank)

T = 298.15
K_B = 1.380649e-23
H = 6.62607015e-34
R = 0.008314462618
EYRING_PREFACTOR = K_B * T / H
RT = R * T
INV_RT = float(np.float32(1.0 / RT))  # reference casts 1/RT to f32
LN_PREF = float(np.log(EYRING_PREFACTOR))

_cached = {}


def _build_program(m_out, bias_val):
    from concourse import bacc, mybir
    from concourse.tile import TileContext

    nc = bacc.Bacc(
        None, target_bir_lowering=False, debug=False, num_devices=N_CORES
    )
    # W is zero-padded to 128 columns: Fast Weight Load only engages at
    # exactly 128 weight columns, and the extra PSUM partitions are free
    # (ACT cost depends on free dim only; pad rows are never DMA'd out).
    MP = 128
    x = nc.dram_tensor("x", [K, BC], mybir.dt.float16, kind="ExternalInput")
    w = nc.dram_tensor("w", [K, MP], mybir.dt.float16, kind="ExternalInput")
    y = nc.dram_tensor("y", [m_out, BC], mybir.dt.bfloat16, kind="ExternalOutput")

    exp = mybir.ActivationFunctionType.Exp

    with TileContext(nc) as tc:
        # HWDGE fans a DMA's descriptors over the largest divisor of the
        # descriptor (row) count that is <= 16 SDMA engines, so DMAs are
        # issued in row-chunks whose count divides by 16 where possible.
        m16 = (m_out // 16) * 16
        with (
            tc.tile_pool(name="consts", bufs=1) as cpool,
            tc.tile_pool(name="inp", bufs=2) as ipool,
            tc.tile_pool(name="outp", bufs=2) as opool,
            tc.tile_pool(name="psum", bufs=2, space="PSUM") as ppool,
        ):
            wt = cpool.tile([K, MP], mybir.dt.float16)
            nc.sync.dma_start(wt[:], w[:])
            bias_t = cpool.tile([128, 1], mybir.dt.float32)
            nc.vector.memset(bias_t[:], bias_val)

            for t in range(BC // F_SUPER):
                c0 = t * F_SUPER
                xt = ipool.tile([K, F_SUPER], mybir.dt.float16, name="xt", tag="xt")
                nc.sync.dma_start(xt[:], x[:, c0 : c0 + F_SUPER])
                out = opool.tile(
                    [MP, F_SUPER], mybir.dt.bfloat16, name="out", tag="out"
                )
                for p in range(F_SUPER // F_PSUM):
                    ps = ppool.tile([MP, F_PSUM], mybir.dt.float32, name="ps", tag="ps")
                    for m in range(F_PSUM // F_MM):
                        a = p * F_PSUM + m * F_MM
                        s = slice(m * F_MM, (m + 1) * F_MM)
                        nc.tensor.matmul(
                            ps[:, s], wt[:], xt[:, a : a + F_MM],
                            start=True, stop=True,
                        )
                    po = slice(p * F_PSUM, (p + 1) * F_PSUM)
                    nc.scalar.activation(
                        out[:, po], ps[:],
                        exp, bias=bias_t[:], scale=INV_RT,
                    )
                cs = slice(c0, c0 + F_SUPER)
                if m16:
                    nc.sync.dma_start(y[:m16, cs], out[:m16, :])
                if m16 < m_out:
                    nc.sync.dma_start(y[m16:, cs], out[m16:, :])
    nc.compile()
    return nc


def _host_prep(state_energies, barrier_energies, from_idx, to_idx, reversible):
    se = np.asarray(state_energies, dtype=np.float32)
    be = np.asarray(barrier_energies, dtype=np.float32)
    fi = np.asarray(from_idx).astype(np.int64)
    ti = np.asarray(to_idx).astype(np.int64)
    rv = np.asarray(reversible).astype(bool)

    # Shift barriers by their (rounded) mean so fp16 keeps ~4 more
    # absolute bits; folded back exactly through the activation bias.
    c_shift = float(np.round(np.float64(be[:4096].mean())))

    x = np.empty((K, B), np.float16)
    x[0:NS] = se.T
    x[NS:] = (be - np.float32(c_shift)).T

    rev_idx = np.flatnonzero(rv)  # transitions with a reverse rate
    n_rev = len(rev_idx)
    m_out = NT + n_rev

    w = np.zeros((K, 128), np.float16)
    cols = np.arange(NT)
    w[fi, cols] = 1.0
    w[NS + cols, cols] = -1.0
    if n_rev:
        rcols = NT + np.arange(n_rev)
        w[ti[rev_idx], rcols] = 1.0
        w[NS + rev_idx, rcols] = -1.0
    bias_val = LN_PREF - c_shift * INV_RT
    return x, w, rev_idx, m_out, bias_val


last_results = None


def kernel(state_energies, barrier_energies, from_idx, to_idx, reversible):
    global last_results
    from concourse.bass_utils import run_bass_kernel_spmd

    x, w, rev_idx, m_out, bias_val = _host_prep(
        state_energies, barrier_energies, from_idx, to_idx, reversible
    )

    key = (m_out, bias_val)
    if key not in _cached:
        _cached[key] = _build_program(m_out, bias_val)
    nc = _cached[key]

    in_maps = []
    for c in range(N_CORES):
        sl = slice(c * BC, (c + 1) * BC)
        in_maps.append({"x": np.ascontiguousarray(x[:, sl]), "w": w})

    res = run_bass_kernel_spmd(
        nc,
        in_maps,
        core_ids=list(range(N_CORES)),
        trace=bool(int(os.environ.get("KERNEL_TRACE", "0"))),
    )
    last_results = res

    n_rev = len(rev_idx)
    forward = np.empty((B, NT), np.float32)
    reverse = np.zeros((B, NT), np.float32)
    for c, r in enumerate(res.results):
        yc = np.asarray(r["y"])
        # bf16 -> f32 via bit shift (exact, faster than astype)
        yf = (yc.view(np.uint16).astype(np.uint32) << 16).view(np.float32)
        forward[c * BC : (c + 1) * BC] = yf[:NT].T
        if n_rev:
            reverse[c * BC : (c + 1) * BC][:, rev_idx] = yf[NT:].T
    return forward, reverse
